# revision 23
# baseline (speedup 1.0000x reference)
"""Trainium2 Bass kernel for nn_CaptioningRNN (attention LSTM over T=64).

Data-parallel over the batch: N=256 samples split across 8 NeuronCores
(32 samples/core), weights replicated, no collectives.

v2 structure (all matmuls bf16 on the TensorEngine, state in f32):
  - tanh-only gates: sigmoid(x) = (tanh(x/2)+1)/2 with the 1/2 folded into
    host-side weight pre-scaling, and h tracked as h2 = 2h (Wh pre-scaled by
    an extra 1/2, score scale halved, output halved on the host). exp and
    tanh live in the same ACT table -> no ACT_TABLE_LOAD per step.
  - xproj phase: xpT = x @ Wx + b computed Wx-stationary, stored to a DRAM
    scratch in bf16 laid out [T, 128, 2, (q,j,n)] so the per-step slice is
    one clean [128, 1024] DMA and doubles as the moving operand of an
    identity-stationary matmul that injects xproj into the transposed-gates
    PSUM accumulation (no DVE adds).
  - P phase: P[n,k,:] = A[n,:,k] @ Wattn precomputed once; h0 = c0 = mean.
  - Recurrence (64 steps): col-tiled scores (4 groups x 2 chunks) ->
    mask+reduce diag extract -> softmax (exp) -> 32x32 transpose -> one-hot
    matmul + broadcast mask mul -> block-diag w; gates = h2 @ Wh' +
    sum_k w_k P'_k in 2 column-tiled PSUM strips (shared stationary across
    the 4 col-groups); strips cast to bf16 on the Scalar engine, transposed
    on PE with the xproj inject; one Tanh per strip; cell math in
    h-on-partition space with the sigmoid affine folded in.
  - Output written [t][p][(r,q,n)] f32; host reassembles + scales by 0.5.
"""

import numpy as np
import ml_dtypes

import concourse.bacc as bacc
import concourse.mybir as mybir
from concourse import bass_utils
from concourse.tile import TileContext

F32, BF16 = mybir.dt.float32, mybir.dt.bfloat16
AF = mybir.ActivationFunctionType
ALU = mybir.AluOpType
AX = mybir.AxisListType
BF = ml_dtypes.bfloat16

N, T, D, H = 256, 64, 1024, 1024
NCORES = 8
NL = N // NCORES          # 32 samples per core
HC = 8                    # 128-row chunks of D/H
G, GS = 4, 8              # sample groups of 8 (for the (k, n_g) 128-partition layout)
H4 = 4 * H                # 4096 gate columns

_built = None


def _consts():
    # E16[k', 8k + n] = (k' == k): one-hot expansion of wT rows onto the
    # (k-major, n_g-minor) 128-partition layout.
    e16 = np.zeros((16, 128), dtype=BF)
    for k in range(16):
        e16[k, 8 * k : 8 * k + 8] = 1
    # M32[p, 32 g + m] = (m % 8 == p % 8) & (m // 8 == g): block-diagonal
    # mask producing masked_g = w[m, k(p)] only for group-g samples.
    p = np.arange(128)[:, None]
    m = np.arange(32)[None, :]
    m32 = np.zeros((128, 128), dtype=BF)
    for g in range(4):
        m32[:, 32 * g : 32 * g + 32] = ((m % 8 == p % 8) & (m // 8 == g)).astype(BF)
    # Mdiag4[32 j + n, 32 k + n'] = (n == n') / 64: extracts the diagonal of
    # the cross-sample score products; 1/64 = softmax scale 1/sqrt(H) times
    # the 1/2 compensating h2 = 2h. Tiled over the 4 col-tile groups.
    md4 = np.zeros((128, 512), dtype=np.float32)
    n_ = np.arange(32)
    for j in range(4):
        for k in range(16):
            md4[32 * j + n_, 32 * k + n_] = 1.0 / 64.0
    eye_bf = np.eye(128, dtype=BF)
    eye_f32 = np.eye(128, dtype=np.float32)
    # sel4[32 j + m, m'] = (m == m'): matmul-stationary that sums the four
    # col-tile partition groups of the partial score reductions.
    sel4 = np.tile(np.eye(32, dtype=np.float32), (4, 1))
    return e16, m32, md4, eye_bf, eye_f32, sel4


def _build_nc(t_steps=T):
    nc = bacc.Bacc(trn_type="TRN2", target_bir_lowering=False, debug=False)

    # host-packed inputs (see _prep_shards for layouts)
    ap_xT = nc.dram_tensor("xTP", [128, HC * T * NL], BF16, kind="ExternalInput").ap()
    ap_Asc = nc.dram_tensor("AscP", [128, HC * 512], BF16, kind="ExternalInput").ap()
    ap_Wx = nc.dram_tensor("WxP", [128, 32 * HC * 128], BF16, kind="ExternalInput").ap()
    ap_Wh = nc.dram_tensor("WhP", [128, HC * H4], BF16, kind="ExternalInput").ap()
    ap_Wa = nc.dram_tensor("WaP", [128, 8 * HC * 512], BF16, kind="ExternalInput").ap()
    ap_bT = nc.dram_tensor("bT", [128, 32], F32, kind="ExternalInput").ap()
    # out2[t, p, r, q, n] = h2[t][n, r*512 + q*128 + p]  (host scales by 0.5)
    out2 = nc.dram_tensor("out2", [T, 128, 2, 4, NL], F32, kind="ExternalOutput").ap()
    # xps[t, p, r, q, j, n] = xproj[t][n, j*1024 + r*512 + q*128 + p]  (bf16)
    xps = nc.dram_tensor("xps", [T, 128, 2, 4, 4, NL], BF16, kind="Internal").ap()

    e16_np, m32_np, md4_np, eye_np, eye32_np, sel4_np = _consts()
    e16_d = nc.inline_tensor(e16_np, "c_e16")
    m32_d = nc.inline_tensor(m32_np, "c_m32")
    md4_d = nc.inline_tensor(md4_np, "c_md4")
    eye_d = nc.inline_tensor(eye_np, "c_eyebf")
    eye32_d = nc.inline_tensor(eye32_np, "c_eye32")
    sel4_d = nc.inline_tensor(sel4_np, "c_sel4")

    with TileContext(nc) as tc:
        with tc.tile_pool(name="pers", bufs=1) as pers:
            Wh_sb = pers.tile([128, HC * H4], BF16, tag="Wh")
            Asc_sb = pers.tile([128, HC * 512], BF16, tag="Asc")
            P_sb = pers.tile([128, G * H4], BF16, tag="P")
            uThA = pers.tile([128, 128], BF16, tag="uThA")
            uThB = pers.tile([128, 128], BF16, tag="uThB")
            cT = pers.tile([128, 256], F32, tag="cT")
            eye = pers.tile([128, 128], BF16, tag="eye")
            eye32 = pers.tile([128, 128], F32, tag="eye32")
            sel4 = pers.tile([128, 32], F32, tag="sel4")
            E16 = pers.tile([16, 128], BF16, tag="E16")
            M32 = pers.tile([128, 128], BF16, tag="M32")
            Mdiag4 = pers.tile([128, 512], F32, tag="Mdiag4")
            b_sb = pers.tile([128, 32], F32, tag="bT")
            wsq = pers.tile([32, 32], BF16, tag="wsq")

            nc.sync.dma_start(eye[:], eye_d.ap()[:])
            nc.sync.dma_start(eye32[:], eye32_d.ap()[:])
            nc.sync.dma_start(sel4[:], sel4_d.ap()[:])
            nc.sync.dma_start(E16[:], e16_d.ap()[:])
            nc.sync.dma_start(M32[:], m32_d.ap()[:])
            nc.sync.dma_start(Mdiag4[:], md4_d.ap()[:])
            nc.sync.dma_start(b_sb[:], ap_bT[:])
            nc.gpsimd.memset(wsq[:], 0.0)
            nc.sync.dma_start(Wh_sb[:], ap_Wh[:])
            nc.sync.dma_start(Asc_sb[:], ap_Asc[:])

            # ------------- phase B: P precompute + h0/c0 init -------------
            with tc.tile_pool(name="php", bufs=3) as php, \
                 tc.tile_pool(name="php1", bufs=1) as php1, \
                 tc.tile_pool(name="psP", bufs=2, space="PSUM") as psP:
                for c in range(HC):
                    h0s = php.tile([128, 32], F32, tag="h0s")
                    nc.vector.tensor_reduce(
                        h0s[:],
                        Asc_sb[:, c * 512 : (c + 1) * 512].rearrange(
                            "p (k n) -> p n k", k=16
                        ),
                        axis=AX.X,
                        op=ALU.add,
                    )
                    nc.vector.tensor_scalar_mul(
                        cT[:, 32 * c : 32 * (c + 1)], h0s[:], 1.0 / 16.0
                    )
                    # uTh holds h2 = 2h -> init 2/16
                    uT = uThA if c < 4 else uThB
                    nc.vector.tensor_scalar_mul(
                        uT[:, 32 * (c % 4) : 32 * (c % 4 + 1)], h0s[:], 2.0 / 16.0
                    )
                # contiguous staging of the group-selected A columns so the
                # matmul stationary operand has a single free dim
                Ag = php1.tile([128, G * HC * 128], BF16, tag="Ag")
                for g in range(G):
                    for c in range(HC):
                        nc.vector.tensor_copy(
                            Ag[:, (g * HC + c) * 128 : (g * HC + c) * 128 + 128],
                            Asc_sb[:, c * 512 : (c + 1) * 512].rearrange(
                                "p (k n) -> p k n", k=16
                            )[:, :, GS * g : GS * (g + 1)],
                        )
                for blk in range(8):
                    Wab = php.tile([128, HC * 512], BF16, tag="Wab")
                    nc.sync.dma_start(
                        Wab[:], ap_Wa[:, blk * HC * 512 : (blk + 1) * HC * 512]
                    )
                    for g in range(G):
                        psp = psP.tile([128, 512], F32, tag="psp")
                        for c in range(HC):
                            nc.tensor.matmul(
                                psp[:],
                                Ag[:, (g * HC + c) * 128 : (g * HC + c) * 128 + 128],
                                Wab[:, c * 512 : (c + 1) * 512],
                                start=(c == 0),
                                stop=(c == HC - 1),
                            )
                        nc.scalar.copy(
                            P_sb[:, g * H4 + 512 * blk : g * H4 + 512 * (blk + 1)],
                            psp[:],
                        )

            # ---------------- phase A: xproj -> DRAM scratch ----------------
            with tc.tile_pool(name="phx1", bufs=1) as phx1, \
                 tc.tile_pool(name="phx", bufs=3) as phx, \
                 tc.tile_pool(name="psX", bufs=2, space="PSUM") as psX:
                xT_sb = phx1.tile([128, HC * T * NL], BF16, tag="xTsb")
                nc.sync.dma_start(xT_sb[:], ap_xT[:])
                for W in range(32):
                    j, r, q = W // 8, (W % 8) // 4, W % 4
                    Wxb = phx.tile([128, HC * 128], BF16, tag="Wxb")
                    nc.sync.dma_start(
                        Wxb[:], ap_Wx[:, W * HC * 128 : (W + 1) * HC * 128]
                    )
                    sxp4 = phx.tile([128, T * NL], BF16, tag="sxp4")
                    for t4 in range(4):
                        psx = psX.tile([128, 512], F32, tag="psx")
                        for c in range(HC):
                            nc.tensor.matmul(
                                psx[:],
                                Wxb[:, c * 128 : (c + 1) * 128],
                                xT_sb[:, c * T * NL + 512 * t4 : c * T * NL + 512 * (t4 + 1)],
                                start=(c == 0),
                                stop=(c == HC - 1),
                            )
                        nc.vector.tensor_scalar_add(
                            sxp4[:, 512 * t4 : 512 * (t4 + 1)], psx[:],
                            b_sb[:, W : W + 1],
                        )
                    # descriptor-heavy scatter writes: round-robin over the
                    # otherwise-idle gpsimd/scalar queues
                    weng = nc.gpsimd if (W % 2 == 0) else nc.scalar
                    weng.dma_start(
                        xps[:, :, r, q, j, :].transpose([1, 0, 2]),
                        sxp4[:].rearrange("p (t n) -> p t n", t=T),
                    )

            # ---------------------- phase C: recurrence ----------------------
            with tc.tile_pool(name="wrk", bufs=2) as wrk, \
                 tc.tile_pool(name="psc", bufs=2, space="PSUM") as psc_pool, \
                 tc.tile_pool(name="pwx", bufs=1, space="PSUM") as pwx_pool, \
                 tc.tile_pool(name="pstr", bufs=1, space="PSUM") as pstr_pool, \
                 tc.tile_pool(name="paT", bufs=2, space="PSUM") as paT_pool:
                for t in range(t_steps):
                    # prefetched xproj slice: [128, (r, q, j, n)] bf16
                    xpt = wrk.tile([128, 1024], BF16, tag="xpt", name=f"xpt_{t}")
                    nc.sync.dma_start(
                        xpt[:].rearrange("p (r q j n) -> p r q j n", r=2, q=4, j=4),
                        xps[t],
                    )

                    # -- scores: col-tiled cross-sample products, diag, softmax
                    psc4 = psc_pool.tile([128, 512], F32, tag="psc4")
                    for c in range(HC):
                        jj, e = c % 4, c // 4
                        uT = uThA if c < 4 else uThB
                        nc.tensor.matmul(
                            psc4[32 * jj : 32 * (jj + 1), :],
                            uT[:, 32 * (c % 4) : 32 * (c % 4) + 32],
                            Asc_sb[:, c * 512 : (c + 1) * 512],
                            start=(e == 0),
                            stop=(e == 1),
                            skip_group_check=True,
                            tile_position=(0, 32 * jj),
                        )
                    scm4 = wrk.tile([128, 512], F32, tag="scm4")
                    nc.vector.tensor_mul(scm4[:], psc4[:], Mdiag4[:])
                    red4 = wrk.tile([128, 16], F32, tag="red4")
                    nc.vector.tensor_reduce(
                        red4[:],
                        scm4[:].rearrange("p (k n) -> p k n", k=16),
                        axis=AX.X,
                        op=ALU.add,
                    )
                    scores = pwx_pool.tile([32, 16], F32, tag="scps",
                                           name=f"scps_{t}")
                    nc.tensor.matmul(
                        scores[:], sel4[:], red4[:], start=True, stop=True
                    )
                    nmx = wrk.tile([32, 1], F32, tag="nmx")
                    nc.vector.tensor_reduce(
                        nmx[:], scores[:], axis=AX.X, op=ALU.max, negate=True
                    )
                    ex = wrk.tile([32, 16], F32, tag="ex")
                    esum = wrk.tile([32, 1], F32, tag="esum")
                    nc.scalar.activation(
                        ex[:], scores[:], AF.Exp, bias=nmx[:], scale=1.0,
                        accum_out=esum[:],
                    )
                    rcp = wrk.tile([32, 1], F32, tag="rcp")
                    nc.vector.reciprocal(rcp[:], esum[:])
                    nc.vector.tensor_scalar_mul(wsq[:, 0:16], ex[:], rcp[:])
                    wT = wrk.tile([32, 32], BF16, tag="wT")
                    nc.vector.transpose(wT[:], wsq[:])
                    pwx = pwx_pool.tile([128, 32], F32, tag="pwx")
                    nc.tensor.matmul(
                        pwx[:], E16[:], wT[0:16, 0:32], start=True, stop=True
                    )
                    masked = wrk.tile([128, 128], BF16, tag="masked")
                    nc.vector.tensor_mul(
                        masked[:].rearrange("p (g m) -> p g m", g=4),
                        pwx[:].rearrange("p (x m) -> p x m", x=1).broadcast_to(
                            [128, 4, 32]
                        ),
                        M32[:].rearrange("p (g m) -> p g m", g=4),
                    )

                    # -- gates: h2 @ Wh' + sum_k w_k P'_k, one strip per r.
                    # PE emission order keeps the array busy while each r's
                    # ACT/DVE tail runs: Wh0+P0, Wh1 (sg0 copies on Scalar),
                    # inject+transpose 0, P1 (cell 0 on DVE), inject+
                    # transpose 1 (cell 1 overlaps next step's scores).
                    h2all = wrk.tile([128, 256], F32, tag="h2all", name=f"h2_{t}")

                    def wh_block(strip, r):
                        for c in range(HC):
                            uT = uThA if c < 4 else uThB
                            for jj in range(4):
                                nc.tensor.matmul(
                                    strip[32 * jj : 32 * (jj + 1), :],
                                    uT[:, 32 * (c % 4) : 32 * (c % 4) + 32],
                                    Wh_sb[:, c * H4 + jj * 1024 + r * 512 : c * H4 + jj * 1024 + r * 512 + 512],
                                    start=(c == 0),
                                    stop=False,
                                    skip_group_check=True,
                                    tile_position=(0, 32 * jj),
                                )

                    def p_block(strip, r):
                        for g in range(G):
                            for jj in range(4):
                                nc.tensor.matmul(
                                    strip[32 * jj : 32 * (jj + 1), :],
                                    masked[:, 32 * g : 32 * g + 32],
                                    P_sb[:, g * H4 + jj * 1024 + r * 512 : g * H4 + jj * 1024 + r * 512 + 512],
                                    start=False,
                                    stop=(g == G - 1),
                                    skip_group_check=True,
                                    tile_position=(0, 32 * jj),
                                )

                    def sg_copy(strip, r):
                        sg = wrk.tile([128, 512], F32, tag=f"sg{r}")
                        nc.scalar.copy(sg[:], strip[:])
                        return sg

                    def transpose_block(sg, r):
                        pat = paT_pool.tile([128, 512], F32, tag="pat",
                                            name=f"pat{r}_{t}")
                        nc.tensor.matmul(
                            pat[:], eye[:],
                            xpt[:, r * 512 : (r + 1) * 512],
                            start=True, stop=False,
                        )
                        for q in range(4):
                            nc.tensor.matmul(
                                pat[:, 128 * q : 128 * (q + 1)],
                                sg[:, 128 * q : 128 * (q + 1)],
                                eye32[:],
                                is_transpose=True,
                                start=False,
                                stop=(q == 3),
                            )
                        return pat

                    def cell_block(pat, r):
                        # tv = tanh over the whole 512 (i/f/o pre-halved)
                        tv = wrk.tile([128, 512], F32, tag=f"tv{r}")
                        nc.scalar.activation(tv[:], pat[:], AF.Tanh)
                        tq = tv[:].rearrange("p (q j m) -> p q j m", q=4, j=4)
                        ti, tf = tq[:, :, 0, :], tq[:, :, 1, :]
                        to, tg = tq[:, :, 2, :], tq[:, :, 3, :]
                        cview = cT[:, 128 * r : 128 * (r + 1)].rearrange(
                            "p (q n) -> p q n", q=4
                        )
                        u = wrk.tile([128, 128], F32, tag=f"u{r}")
                        nc.vector.scalar_tensor_tensor(
                            u[:].rearrange("p (q n) -> p q n", q=4),
                            tf, 1.0, cview, ALU.add, ALU.mult,
                        )
                        v = wrk.tile([128, 128], F32, tag=f"v{r}")
                        nc.vector.scalar_tensor_tensor(
                            v[:].rearrange("p (q n) -> p q n", q=4),
                            ti, 1.0, tg, ALU.add, ALU.mult,
                        )
                        s2 = wrk.tile([128, 128], F32, tag=f"s2{r}")
                        nc.vector.tensor_add(s2[:], u[:], v[:])
                        nc.vector.tensor_scalar_mul(
                            cT[:, 128 * r : 128 * (r + 1)], s2[:], 0.5
                        )
                        th = wrk.tile([128, 128], F32, tag=f"th{r}")
                        nc.scalar.activation(
                            th[:], cT[:, 128 * r : 128 * (r + 1)], AF.Tanh
                        )
                        # h2 = (to + 1) * tanh(c)
                        nc.vector.scalar_tensor_tensor(
                            h2all[:, 128 * r : 128 * (r + 1)].rearrange(
                                "p (q n) -> p q n", q=4
                            ),
                            to, 1.0, th[:].rearrange("p (q n) -> p q n", q=4),
                            ALU.add, ALU.mult,
                        )
                        uT = uThA if r == 0 else uThB
                        nc.vector.tensor_copy(
                            uT[:], h2all[:, 128 * r : 128 * (r + 1)]
                        )

                    strip0 = pstr_pool.tile([128, 512], F32, tag="strip0",
                                            name=f"strip0_{t}")
                    strip1 = pstr_pool.tile([128, 512], F32, tag="strip1",
                                            name=f"strip1_{t}")
                    wh_block(strip0, 0)
                    p_block(strip0, 0)
                    sg0 = sg_copy(strip0, 0)
                    wh_block(strip1, 1)
                    pat0 = transpose_block(sg0, 0)
                    p_block(strip1, 1)
                    sg1 = sg_copy(strip1, 1)
                    pat1 = transpose_block(sg1, 1)
                    cell_block(pat0, 0)
                    cell_block(pat1, 1)
                    nc.sync.dma_start(
                        out2[t],
                        h2all[:].rearrange("p (r q n) -> p r q n", r=2, q=4),
                    )
    nc.compile()
    return nc


def _prep_shards(inputs):
    x = np.asarray(inputs["x"], np.float32)
    A = np.asarray(inputs["A"], np.float32)
    Wx = np.asarray(inputs["Wx"], np.float32)
    Wh = np.asarray(inputs["Wh"], np.float32)
    Wattn = np.asarray(inputs["Wattn"], np.float32)
    b = np.asarray(inputs["b"], np.float32)

    # tanh-only gate scaling: i/f/o columns get the sigmoid 1/2 arg-scale;
    # everything fed by h2 = 2h gets an extra 1/2.
    sc_ifo = np.ones((H4,), np.float32)
    sc_ifo[: 3 * H] = 0.5
    Wx_s = Wx * sc_ifo
    b_s = b * sc_ifo
    Wa_s = Wattn * sc_ifo
    Wh_s = Wh * (0.5 * sc_ifo)

    def chunk_rows(M, free):
        # [1024, F] -> [128, HC * F] with the 8 row-chunks along free
        return np.ascontiguousarray(
            M.reshape(HC, 128, free).transpose(1, 0, 2).reshape(128, HC * free)
        )

    Wh_bf = chunk_rows(Wh_s.astype(BF), H4)
    # WxP: [128, (W, c, 128)]
    WxP = np.ascontiguousarray(
        Wx_s.astype(BF)
        .reshape(HC, 128, 32, 128)
        .transpose(1, 2, 0, 3)
        .reshape(128, 32 * HC * 128)
    )
    # WaP: [128, (blk, c, 512)]
    WaP = np.ascontiguousarray(
        Wa_s.astype(BF)
        .reshape(HC, 128, 8, 512)
        .transpose(1, 2, 0, 3)
        .reshape(128, 8 * HC * 512)
    )
    bT = np.ascontiguousarray(b_s.reshape(32, 128).T.astype(np.float32))

    in_maps = []
    for i in range(NCORES):
        ns = slice(NL * i, NL * (i + 1))
        xT = x[ns].transpose(2, 1, 0).reshape(D, T * NL)
        xTP = chunk_rows(np.ascontiguousarray(xT).astype(BF), T * NL)
        Asc = A[ns].reshape(NL, H, 16).transpose(1, 2, 0).reshape(H, 512)
        AscP = chunk_rows(np.ascontiguousarray(Asc).astype(BF), 512)
        in_maps.append(
            {
                "xTP": xTP,
                "AscP": AscP,
                "WxP": WxP,
                "WhP": Wh_bf,
                "WaP": WaP,
                "bT": bT,
            }
        )
    return in_maps


def _get_nc():
    global _built
    if _built is None:
        _built = _build_nc()
    return _built


def _run(inputs, **kwargs):
    nc = _get_nc()
    in_maps = _prep_shards(inputs)
    res = bass_utils.run_bass_kernel_spmd(
        nc, in_maps, core_ids=list(range(NCORES)), **kwargs
    )
    out = np.empty((N, T, H), np.float32)
    for i in range(NCORES):
        o2 = res.results[i]["out2"]  # [T, 128, 2, 4, NL], h2 values
        out[NL * i : NL * (i + 1)] = 0.5 * o2.transpose(4, 0, 2, 3, 1).reshape(
            NL, T, H
        )
    return out, res


def kernel(**inputs):
    out, _ = _run(inputs)
    return out


# revision 24
# speedup vs baseline: 1.0609x; 1.0609x over previous
"""Trainium2 Bass kernel for nn_CaptioningRNN (attention LSTM over T=64).

Data-parallel over the batch: N=256 samples split across 8 NeuronCores
(32 samples/core), weights replicated, no collectives.

v2 structure (all matmuls bf16 on the TensorEngine, state in f32):
  - tanh-only gates: sigmoid(x) = (tanh(x/2)+1)/2 with the 1/2 folded into
    host-side weight pre-scaling, and h tracked as h2 = 2h (Wh pre-scaled by
    an extra 1/2, score scale halved, output halved on the host). exp and
    tanh live in the same ACT table -> no ACT_TABLE_LOAD per step.
  - xproj phase: xpT = x @ Wx + b computed Wx-stationary, stored to a DRAM
    scratch in bf16 laid out [T, 128, 2, (q,j,n)] so the per-step slice is
    one clean [128, 1024] DMA and doubles as the moving operand of an
    identity-stationary matmul that injects xproj into the transposed-gates
    PSUM accumulation (no DVE adds).
  - P phase: P[n,k,:] = A[n,:,k] @ Wattn precomputed once; h0 = c0 = mean.
  - Recurrence (64 steps): col-tiled scores (4 groups x 2 chunks) ->
    mask+reduce diag extract -> softmax (exp) -> 32x32 transpose -> one-hot
    matmul + broadcast mask mul -> block-diag w; gates = h2 @ Wh' +
    sum_k w_k P'_k in 2 column-tiled PSUM strips (shared stationary across
    the 4 col-groups); strips cast to bf16 on the Scalar engine, transposed
    on PE with the xproj inject; one Tanh per strip; cell math in
    h-on-partition space with the sigmoid affine folded in.
  - Output written [t][p][(r,q,n)] f32; host reassembles + scales by 0.5.
"""

import numpy as np
import ml_dtypes

import concourse.bacc as bacc
import concourse.mybir as mybir
from concourse import bass_utils
from concourse.tile import TileContext

F32, BF16 = mybir.dt.float32, mybir.dt.bfloat16
AF = mybir.ActivationFunctionType
ALU = mybir.AluOpType
AX = mybir.AxisListType
BF = ml_dtypes.bfloat16

N, T, D, H = 256, 64, 1024, 1024
NCORES = 8
NL = N // NCORES          # 32 samples per core
HC = 8                    # 128-row chunks of D/H
G, GS = 4, 8              # sample groups of 8 (for the (k, n_g) 128-partition layout)
H4 = 4 * H                # 4096 gate columns

_built = None


def _consts():
    # E16[k', 8k + n] = (k' == k): one-hot expansion of wT rows onto the
    # (k-major, n_g-minor) 128-partition layout.
    e16 = np.zeros((16, 128), dtype=BF)
    for k in range(16):
        e16[k, 8 * k : 8 * k + 8] = 1
    # M32[p, 32 g + m] = (m % 8 == p % 8) & (m // 8 == g): block-diagonal
    # mask producing masked_g = w[m, k(p)] only for group-g samples.
    p = np.arange(128)[:, None]
    m = np.arange(32)[None, :]
    m32 = np.zeros((128, 128), dtype=BF)
    for g in range(4):
        m32[:, 32 * g : 32 * g + 32] = ((m % 8 == p % 8) & (m // 8 == g)).astype(BF)
    # Mdiag4[32 j + n, 32 k + n'] = (n == n') / 64: extracts the diagonal of
    # the cross-sample score products; 1/64 = softmax scale 1/sqrt(H) times
    # the 1/2 compensating h2 = 2h. Tiled over the 4 col-tile groups.
    md4 = np.zeros((128, 512), dtype=np.float32)
    n_ = np.arange(32)
    for j in range(4):
        for k in range(16):
            md4[32 * j + n_, 32 * k + n_] = 1.0 / 64.0
    eye_bf = np.eye(128, dtype=BF)
    eye_f32 = np.eye(128, dtype=np.float32)
    # sel4[32 j + m, m'] = (m == m'): matmul-stationary that sums the four
    # col-tile partition groups of the partial score reductions.
    sel4 = np.tile(np.eye(32, dtype=np.float32), (4, 1))
    return e16, m32, md4, eye_bf, eye_f32, sel4


def _build_nc(t_steps=T):
    nc = bacc.Bacc(trn_type="TRN2", target_bir_lowering=False, debug=False)

    # host-packed inputs (see _prep_shards for layouts)
    ap_xT = nc.dram_tensor("xTP", [128, HC * T * NL], BF16, kind="ExternalInput").ap()
    ap_Asc = nc.dram_tensor("AscP", [128, HC * 512], BF16, kind="ExternalInput").ap()
    ap_Wx = nc.dram_tensor("WxP", [128, 32 * HC * 128], BF16, kind="ExternalInput").ap()
    ap_Wh = nc.dram_tensor("WhP", [128, HC * H4], BF16, kind="ExternalInput").ap()
    ap_Wa = nc.dram_tensor("WaP", [128, 8 * HC * 512], BF16, kind="ExternalInput").ap()
    ap_bT = nc.dram_tensor("bT", [128, 32], F32, kind="ExternalInput").ap()
    # out2[t, p, r, q, n] = h2[t][n, r*512 + q*128 + p]  (host scales by 0.5)
    out2 = nc.dram_tensor("out2", [T, 128, 2, 4, NL], F32, kind="ExternalOutput").ap()
    # xps[t, p, r, q, j, n] = xproj[t][n, j*1024 + r*512 + q*128 + p]  (bf16)
    xps = nc.dram_tensor("xps", [T, 128, 2, 4, 4, NL], BF16, kind="Internal").ap()

    e16_np, m32_np, md4_np, eye_np, eye32_np, sel4_np = _consts()
    e16_d = nc.inline_tensor(e16_np, "c_e16")
    m32_d = nc.inline_tensor(m32_np, "c_m32")
    md4_d = nc.inline_tensor(md4_np, "c_md4")
    eye_d = nc.inline_tensor(eye_np, "c_eyebf")
    eye32_d = nc.inline_tensor(eye32_np, "c_eye32")
    sel4_d = nc.inline_tensor(sel4_np, "c_sel4")

    with TileContext(nc) as tc:
        with tc.tile_pool(name="pers", bufs=1) as pers:
            Wh_sb = pers.tile([128, HC * H4], BF16, tag="Wh")
            Asc_sb = pers.tile([128, HC * 512], BF16, tag="Asc")
            P_sb = pers.tile([128, G * H4], BF16, tag="P")
            uThA = pers.tile([128, 128], BF16, tag="uThA")
            uThB = pers.tile([128, 128], BF16, tag="uThB")
            cT = pers.tile([128, 256], F32, tag="cT")
            eye = pers.tile([128, 128], BF16, tag="eye")
            eye32 = pers.tile([128, 128], F32, tag="eye32")
            sel4 = pers.tile([128, 32], F32, tag="sel4")
            E16 = pers.tile([16, 128], BF16, tag="E16")
            M32 = pers.tile([128, 128], BF16, tag="M32")
            Mdiag4 = pers.tile([128, 512], F32, tag="Mdiag4")
            b_sb = pers.tile([128, 32], F32, tag="bT")
            wsq = pers.tile([32, 32], BF16, tag="wsq")

            nc.sync.dma_start(eye[:], eye_d.ap()[:])
            nc.sync.dma_start(eye32[:], eye32_d.ap()[:])
            nc.sync.dma_start(sel4[:], sel4_d.ap()[:])
            nc.sync.dma_start(E16[:], e16_d.ap()[:])
            nc.sync.dma_start(M32[:], m32_d.ap()[:])
            nc.sync.dma_start(Mdiag4[:], md4_d.ap()[:])
            nc.sync.dma_start(b_sb[:], ap_bT[:])
            nc.gpsimd.memset(wsq[:], 0.0)
            nc.sync.dma_start(Wh_sb[:], ap_Wh[:])
            nc.sync.dma_start(Asc_sb[:], ap_Asc[:])

            # ------------- phase B: P precompute + h0/c0 init -------------
            with tc.tile_pool(name="php", bufs=3) as php, \
                 tc.tile_pool(name="php1", bufs=1) as php1, \
                 tc.tile_pool(name="psP", bufs=2, space="PSUM") as psP:
                for c in range(HC):
                    h0s = php.tile([128, 32], F32, tag="h0s")
                    nc.vector.tensor_reduce(
                        h0s[:],
                        Asc_sb[:, c * 512 : (c + 1) * 512].rearrange(
                            "p (k n) -> p n k", k=16
                        ),
                        axis=AX.X,
                        op=ALU.add,
                    )
                    nc.vector.tensor_scalar_mul(
                        cT[:, 32 * c : 32 * (c + 1)], h0s[:], 1.0 / 16.0
                    )
                    # uTh holds h2 = 2h -> init 2/16
                    uT = uThA if c < 4 else uThB
                    nc.vector.tensor_scalar_mul(
                        uT[:, 32 * (c % 4) : 32 * (c % 4 + 1)], h0s[:], 2.0 / 16.0
                    )
                # contiguous staging of the group-selected A columns so the
                # matmul stationary operand has a single free dim
                Ag = php1.tile([128, G * HC * 128], BF16, tag="Ag")
                for g in range(G):
                    for c in range(HC):
                        nc.vector.tensor_copy(
                            Ag[:, (g * HC + c) * 128 : (g * HC + c) * 128 + 128],
                            Asc_sb[:, c * 512 : (c + 1) * 512].rearrange(
                                "p (k n) -> p k n", k=16
                            )[:, :, GS * g : GS * (g + 1)],
                        )
                for blk in range(8):
                    Wab = php.tile([128, HC * 512], BF16, tag="Wab")
                    nc.sync.dma_start(
                        Wab[:], ap_Wa[:, blk * HC * 512 : (blk + 1) * HC * 512]
                    )
                    for g in range(G):
                        psp = psP.tile([128, 512], F32, tag="psp")
                        for c in range(HC):
                            nc.tensor.matmul(
                                psp[:],
                                Ag[:, (g * HC + c) * 128 : (g * HC + c) * 128 + 128],
                                Wab[:, c * 512 : (c + 1) * 512],
                                start=(c == 0),
                                stop=(c == HC - 1),
                            )
                        nc.scalar.copy(
                            P_sb[:, g * H4 + 512 * blk : g * H4 + 512 * (blk + 1)],
                            psp[:],
                        )

            # ---------------- phase A: xproj -> DRAM scratch ----------------
            with tc.tile_pool(name="phx1", bufs=1) as phx1, \
                 tc.tile_pool(name="phx", bufs=3) as phx, \
                 tc.tile_pool(name="psX", bufs=2, space="PSUM") as psX:
                xT_sb = phx1.tile([128, HC * T * NL], BF16, tag="xTsb")
                nc.sync.dma_start(xT_sb[:], ap_xT[:])
                for W in range(32):
                    j, r, q = W // 8, (W % 8) // 4, W % 4
                    Wxb = phx.tile([128, HC * 128], BF16, tag="Wxb")
                    nc.sync.dma_start(
                        Wxb[:], ap_Wx[:, W * HC * 128 : (W + 1) * HC * 128]
                    )
                    sxp4 = phx.tile([128, T * NL], BF16, tag="sxp4")
                    for t4 in range(4):
                        psx = psX.tile([128, 512], F32, tag="psx")
                        for c in range(HC):
                            nc.tensor.matmul(
                                psx[:],
                                Wxb[:, c * 128 : (c + 1) * 128],
                                xT_sb[:, c * T * NL + 512 * t4 : c * T * NL + 512 * (t4 + 1)],
                                start=(c == 0),
                                stop=(c == HC - 1),
                            )
                        nc.vector.tensor_scalar_add(
                            sxp4[:, 512 * t4 : 512 * (t4 + 1)], psx[:],
                            b_sb[:, W : W + 1],
                        )
                    # descriptor-heavy scatter writes: round-robin over the
                    # otherwise-idle gpsimd/scalar queues
                    weng = nc.gpsimd if (W % 2 == 0) else nc.scalar
                    weng.dma_start(
                        xps[:, :, r, q, j, :].transpose([1, 0, 2]),
                        sxp4[:].rearrange("p (t n) -> p t n", t=T),
                    )

            # ---------------------- phase C: recurrence ----------------------
            with tc.tile_pool(name="wrk", bufs=2) as wrk, \
                 tc.tile_pool(name="psc", bufs=2, space="PSUM") as psc_pool, \
                 tc.tile_pool(name="pwx", bufs=1, space="PSUM") as pwx_pool, \
                 tc.tile_pool(name="pstr", bufs=1, space="PSUM") as pstr_pool, \
                 tc.tile_pool(name="paT", bufs=2, space="PSUM") as paT_pool:
                for t in range(t_steps):
                    # prefetched xproj slice: [128, (r, q, j, n)] bf16
                    xpt = wrk.tile([128, 1024], BF16, tag="xpt", name=f"xpt_{t}")
                    nc.sync.dma_start(
                        xpt[:].rearrange("p (r q j n) -> p r q j n", r=2, q=4, j=4),
                        xps[t],
                    )

                    # -- scores: col-tiled cross-sample products, diag, softmax
                    psc4 = psc_pool.tile([128, 512], F32, tag="psc4")
                    for c in range(HC):
                        jj, e = c % 4, c // 4
                        uT = uThA if c < 4 else uThB
                        nc.tensor.matmul(
                            psc4[32 * jj : 32 * (jj + 1), :],
                            uT[:, 32 * (c % 4) : 32 * (c % 4) + 32],
                            Asc_sb[:, c * 512 : (c + 1) * 512],
                            start=(e == 0),
                            stop=(e == 1),
                            skip_group_check=True,
                            tile_position=(0, 32 * jj),
                        )
                    scm4 = wrk.tile([128, 512], F32, tag="scm4")
                    nc.vector.tensor_mul(scm4[:], psc4[:], Mdiag4[:])
                    red4 = wrk.tile([128, 16], F32, tag="red4")
                    nc.vector.tensor_reduce(
                        red4[:],
                        scm4[:].rearrange("p (k n) -> p k n", k=16),
                        axis=AX.X,
                        op=ALU.add,
                    )
                    scores = pwx_pool.tile([32, 16], F32, tag="scps",
                                           name=f"scps_{t}")
                    nc.tensor.matmul(
                        scores[:], sel4[:], red4[:], start=True, stop=True
                    )
                    nmx = wrk.tile([32, 1], F32, tag="nmx")
                    nc.vector.tensor_reduce(
                        nmx[:], scores[:], axis=AX.X, op=ALU.max, negate=True
                    )
                    ex = wrk.tile([32, 16], F32, tag="ex")
                    esum = wrk.tile([32, 1], F32, tag="esum")
                    nc.scalar.activation(
                        ex[:], scores[:], AF.Exp, bias=nmx[:], scale=1.0,
                        accum_out=esum[:],
                    )
                    rcp = wrk.tile([32, 1], F32, tag="rcp")
                    nc.vector.reciprocal(rcp[:], esum[:])
                    nc.vector.tensor_scalar_mul(wsq[:, 0:16], ex[:], rcp[:])
                    wT = wrk.tile([32, 32], BF16, tag="wT")
                    nc.vector.transpose(wT[:], wsq[:])
                    pwx = pwx_pool.tile([128, 32], F32, tag="pwx")
                    nc.tensor.matmul(
                        pwx[:], E16[:], wT[0:16, 0:32], start=True, stop=True
                    )
                    masked = wrk.tile([128, 128], BF16, tag="masked")
                    nc.vector.tensor_mul(
                        masked[:].rearrange("p (g m) -> p g m", g=4),
                        pwx[:].rearrange("p (x m) -> p x m", x=1).broadcast_to(
                            [128, 4, 32]
                        ),
                        M32[:].rearrange("p (g m) -> p g m", g=4),
                    )

                    # -- gates: h2 @ Wh' + sum_k w_k P'_k, one strip per r.
                    # PE emission order keeps the array busy while each r's
                    # ACT/DVE tail runs: Wh0+P0, Wh1 (sg0 copies on Scalar),
                    # inject+transpose 0, P1 (cell 0 on DVE), inject+
                    # transpose 1 (cell 1 overlaps next step's scores).
                    h2all = wrk.tile([128, 256], F32, tag="h2all", name=f"h2_{t}")

                    def wh_block(strip, r):
                        for c in range(HC):
                            uT = uThA if c < 4 else uThB
                            for jj in range(4):
                                nc.tensor.matmul(
                                    strip[32 * jj : 32 * (jj + 1), :],
                                    uT[:, 32 * (c % 4) : 32 * (c % 4) + 32],
                                    Wh_sb[:, c * H4 + jj * 1024 + r * 512 : c * H4 + jj * 1024 + r * 512 + 512],
                                    start=(c == 0),
                                    stop=False,
                                    skip_group_check=True,
                                    tile_position=(0, 32 * jj),
                                )

                    def p_block(strip, r):
                        for g in range(G):
                            for jj in range(4):
                                nc.tensor.matmul(
                                    strip[32 * jj : 32 * (jj + 1), :],
                                    masked[:, 32 * g : 32 * g + 32],
                                    P_sb[:, g * H4 + jj * 1024 + r * 512 : g * H4 + jj * 1024 + r * 512 + 512],
                                    start=False,
                                    stop=(g == G - 1),
                                    skip_group_check=True,
                                    tile_position=(0, 32 * jj),
                                )

                    def sg_copy(strip, r):
                        sg = wrk.tile([128, 512], F32, tag=f"sg{r}")
                        nc.scalar.copy(sg[:], strip[:])
                        return sg

                    def transpose_block(sg, r):
                        pat = paT_pool.tile([128, 512], F32, tag="pat",
                                            name=f"pat{r}_{t}")
                        nc.tensor.matmul(
                            pat[:], eye[:],
                            xpt[:, r * 512 : (r + 1) * 512],
                            start=True, stop=False,
                        )
                        for q in range(4):
                            nc.tensor.matmul(
                                pat[:, 128 * q : 128 * (q + 1)],
                                sg[:, 128 * q : 128 * (q + 1)],
                                eye32[:],
                                is_transpose=True,
                                start=False,
                                stop=(q == 3),
                            )
                        return pat

                    def cell_block(pat, r):
                        # tv = tanh over the whole 512 (i/f/o pre-halved)
                        tv = wrk.tile([128, 512], F32, tag=f"tv{r}")
                        nc.scalar.activation(tv[:], pat[:], AF.Tanh)
                        tq = tv[:].rearrange("p (q j m) -> p q j m", q=4, j=4)
                        ti, tf = tq[:, :, 0, :], tq[:, :, 1, :]
                        to, tg = tq[:, :, 2, :], tq[:, :, 3, :]
                        cview = cT[:, 128 * r : 128 * (r + 1)].rearrange(
                            "p (q n) -> p q n", q=4
                        )
                        u = wrk.tile([128, 128], F32, tag=f"u{r}")
                        nc.vector.scalar_tensor_tensor(
                            u[:].rearrange("p (q n) -> p q n", q=4),
                            tf, 1.0, cview, ALU.add, ALU.mult,
                        )
                        v = wrk.tile([128, 128], F32, tag=f"v{r}")
                        nc.vector.scalar_tensor_tensor(
                            v[:].rearrange("p (q n) -> p q n", q=4),
                            ti, 1.0, tg, ALU.add, ALU.mult,
                        )
                        s2 = wrk.tile([128, 128], F32, tag=f"s2{r}")
                        nc.vector.tensor_add(s2[:], u[:], v[:])
                        nc.vector.tensor_scalar_mul(
                            cT[:, 128 * r : 128 * (r + 1)], s2[:], 0.5
                        )
                        th = wrk.tile([128, 128], F32, tag=f"th{r}")
                        nc.scalar.activation(
                            th[:], cT[:, 128 * r : 128 * (r + 1)], AF.Tanh
                        )
                        # h2 = (to + 1) * tanh(c)
                        nc.vector.scalar_tensor_tensor(
                            h2all[:, 128 * r : 128 * (r + 1)].rearrange(
                                "p (q n) -> p q n", q=4
                            ),
                            to, 1.0, th[:].rearrange("p (q n) -> p q n", q=4),
                            ALU.add, ALU.mult,
                        )
                        uT = uThA if r == 0 else uThB
                        nc.vector.tensor_copy(
                            uT[:], h2all[:, 128 * r : 128 * (r + 1)]
                        )

                    strip0 = pstr_pool.tile([128, 512], F32, tag="strip0",
                                            name=f"strip0_{t}")
                    strip1 = pstr_pool.tile([128, 512], F32, tag="strip1",
                                            name=f"strip1_{t}")
                    strips = [strip0, strip1]
                    # interleave the two strips' Wh accumulation (v2 structure:
                    # keeps the PE stream dense), then the P accumulation, then
                    # the per-r tails
                    for c in range(HC):
                        uT = uThA if c < 4 else uThB
                        for r in range(2):
                            for jj in range(4):
                                nc.tensor.matmul(
                                    strips[r][32 * jj : 32 * (jj + 1), :],
                                    uT[:, 32 * (c % 4) : 32 * (c % 4) + 32],
                                    Wh_sb[:, c * H4 + jj * 1024 + r * 512 : c * H4 + jj * 1024 + r * 512 + 512],
                                    start=(c == 0),
                                    stop=False,
                                    skip_group_check=True,
                                    tile_position=(0, 32 * jj),
                                )
                    for r in range(2):
                        p_block(strips[r], r)
                    sg0 = sg_copy(strip0, 0)
                    sg1 = sg_copy(strip1, 1)
                    pat0 = transpose_block(sg0, 0)
                    pat1 = transpose_block(sg1, 1)
                    cell_block(pat0, 0)
                    cell_block(pat1, 1)
                    nc.sync.dma_start(
                        out2[t],
                        h2all[:].rearrange("p (r q n) -> p r q n", r=2, q=4),
                    )
    nc.compile()
    return nc


def _prep_shards(inputs):
    x = np.asarray(inputs["x"], np.float32)
    A = np.asarray(inputs["A"], np.float32)
    Wx = np.asarray(inputs["Wx"], np.float32)
    Wh = np.asarray(inputs["Wh"], np.float32)
    Wattn = np.asarray(inputs["Wattn"], np.float32)
    b = np.asarray(inputs["b"], np.float32)

    # tanh-only gate scaling: i/f/o columns get the sigmoid 1/2 arg-scale;
    # everything fed by h2 = 2h gets an extra 1/2.
    sc_ifo = np.ones((H4,), np.float32)
    sc_ifo[: 3 * H] = 0.5
    Wx_s = Wx * sc_ifo
    b_s = b * sc_ifo
    Wa_s = Wattn * sc_ifo
    Wh_s = Wh * (0.5 * sc_ifo)

    def chunk_rows(M, free):
        # [1024, F] -> [128, HC * F] with the 8 row-chunks along free
        return np.ascontiguousarray(
            M.reshape(HC, 128, free).transpose(1, 0, 2).reshape(128, HC * free)
        )

    Wh_bf = chunk_rows(Wh_s.astype(BF), H4)
    # WxP: [128, (W, c, 128)]
    WxP = np.ascontiguousarray(
        Wx_s.astype(BF)
        .reshape(HC, 128, 32, 128)
        .transpose(1, 2, 0, 3)
        .reshape(128, 32 * HC * 128)
    )
    # WaP: [128, (blk, c, 512)]
    WaP = np.ascontiguousarray(
        Wa_s.astype(BF)
        .reshape(HC, 128, 8, 512)
        .transpose(1, 2, 0, 3)
        .reshape(128, 8 * HC * 512)
    )
    bT = np.ascontiguousarray(b_s.reshape(32, 128).T.astype(np.float32))

    in_maps = []
    for i in range(NCORES):
        ns = slice(NL * i, NL * (i + 1))
        xT = x[ns].transpose(2, 1, 0).reshape(D, T * NL)
        xTP = chunk_rows(np.ascontiguousarray(xT).astype(BF), T * NL)
        Asc = A[ns].reshape(NL, H, 16).transpose(1, 2, 0).reshape(H, 512)
        AscP = chunk_rows(np.ascontiguousarray(Asc).astype(BF), 512)
        in_maps.append(
            {
                "xTP": xTP,
                "AscP": AscP,
                "WxP": WxP,
                "WhP": Wh_bf,
                "WaP": WaP,
                "bT": bT,
            }
        )
    return in_maps


def _get_nc():
    global _built
    if _built is None:
        _built = _build_nc()
    return _built


def _run(inputs, **kwargs):
    nc = _get_nc()
    in_maps = _prep_shards(inputs)
    res = bass_utils.run_bass_kernel_spmd(
        nc, in_maps, core_ids=list(range(NCORES)), **kwargs
    )
    out = np.empty((N, T, H), np.float32)
    for i in range(NCORES):
        o2 = res.results[i]["out2"]  # [T, 128, 2, 4, NL], h2 values
        out[NL * i : NL * (i + 1)] = 0.5 * o2.transpose(4, 0, 2, 3, 1).reshape(
            NL, T, H
        )
    return out, res


def kernel(**inputs):
    out, _ = _run(inputs)
    return out


# revision 28
# speedup vs baseline: 1.1186x; 1.0545x over previous
"""Trainium2 Bass kernel for nn_CaptioningRNN (attention LSTM over T=64).

Data-parallel over the batch: N=256 samples split across 8 NeuronCores
(32 samples/core), weights replicated, no collectives.

v2 structure (all matmuls bf16 on the TensorEngine, state in f32):
  - tanh-only gates: sigmoid(x) = (tanh(x/2)+1)/2 with the 1/2 folded into
    host-side weight pre-scaling, and h tracked as h2 = 2h (Wh pre-scaled by
    an extra 1/2, score scale halved, output halved on the host). exp and
    tanh live in the same ACT table -> no ACT_TABLE_LOAD per step.
  - xproj phase: xpT = x @ Wx + b computed Wx-stationary, stored to a DRAM
    scratch in bf16 laid out [T, 128, 2, (q,j,n)] so the per-step slice is
    one clean [128, 1024] DMA and doubles as the moving operand of an
    identity-stationary matmul that injects xproj into the transposed-gates
    PSUM accumulation (no DVE adds).
  - P phase: P[n,k,:] = A[n,:,k] @ Wattn precomputed once; h0 = c0 = mean.
  - Recurrence (64 steps): col-tiled scores (4 groups x 2 chunks) ->
    mask+reduce diag extract -> softmax (exp) -> 32x32 transpose -> one-hot
    matmul + broadcast mask mul -> block-diag w; gates = h2 @ Wh' +
    sum_k w_k P'_k in 2 column-tiled PSUM strips (shared stationary across
    the 4 col-groups); strips cast to bf16 on the Scalar engine, transposed
    on PE with the xproj inject; one Tanh per strip; cell math in
    h-on-partition space with the sigmoid affine folded in.
  - Output written [t][p][(r,q,n)] f32; host reassembles + scales by 0.5.
"""

import numpy as np
import ml_dtypes

import concourse.bacc as bacc
import concourse.mybir as mybir
from concourse import bass_utils
from concourse.tile import TileContext

F32, BF16 = mybir.dt.float32, mybir.dt.bfloat16
AF = mybir.ActivationFunctionType
ALU = mybir.AluOpType
AX = mybir.AxisListType
BF = ml_dtypes.bfloat16

N, T, D, H = 256, 64, 1024, 1024
NCORES = 8
NL = N // NCORES          # 32 samples per core
HC = 8                    # 128-row chunks of D/H
G, GS = 4, 8              # sample groups of 8 (for the (k, n_g) 128-partition layout)
H4 = 4 * H                # 4096 gate columns

_built = None


def _consts():
    # E16[k', 8k + n] = (k' == k): one-hot expansion of wT rows onto the
    # (k-major, n_g-minor) 128-partition layout.
    e16 = np.zeros((16, 128), dtype=BF)
    for k in range(16):
        e16[k, 8 * k : 8 * k + 8] = 1
    # M32[p, 32 g + m] = (m % 8 == p % 8) & (m // 8 == g): block-diagonal
    # mask producing masked_g = w[m, k(p)] only for group-g samples.
    p = np.arange(128)[:, None]
    m = np.arange(32)[None, :]
    m32 = np.zeros((128, 128), dtype=BF)
    for g in range(4):
        m32[:, 32 * g : 32 * g + 32] = ((m % 8 == p % 8) & (m // 8 == g)).astype(BF)
    # Mdiag4[32 j + n, 32 k + n'] = (n == n') / 64: extracts the diagonal of
    # the cross-sample score products; 1/64 = softmax scale 1/sqrt(H) times
    # the 1/2 compensating h2 = 2h. Tiled over the 4 col-tile groups.
    md4 = np.zeros((128, 512), dtype=np.float32)
    n_ = np.arange(32)
    for j in range(4):
        for k in range(16):
            md4[32 * j + n_, 32 * k + n_] = 1.0 / 64.0
    eye_bf = np.eye(128, dtype=BF)
    eye_f32 = np.eye(128, dtype=np.float32)
    # sel4[32 j + m, m'] = (m == m'): matmul-stationary that sums the four
    # col-tile partition groups of the partial score reductions.
    sel4 = np.tile(np.eye(32, dtype=np.float32), (4, 1))
    return e16, m32, md4, eye_bf, eye_f32, sel4


def _build_nc(t_steps=T):
    nc = bacc.Bacc(trn_type="TRN2", target_bir_lowering=False, debug=False)

    # host-packed inputs (see _prep_shards for layouts)
    ap_xT = nc.dram_tensor("xTP", [128, HC * T * NL], BF16, kind="ExternalInput").ap()
    ap_Asc = nc.dram_tensor("AscP", [128, HC * 512], BF16, kind="ExternalInput").ap()
    ap_Wx = nc.dram_tensor("WxP", [128, 32 * HC * 128], BF16, kind="ExternalInput").ap()
    ap_Wh = nc.dram_tensor("WhP", [128, HC * H4], BF16, kind="ExternalInput").ap()
    ap_Wa = nc.dram_tensor("WaP", [128, 8 * HC * 512], BF16, kind="ExternalInput").ap()
    ap_bT = nc.dram_tensor("bT", [128, 32], F32, kind="ExternalInput").ap()
    # out2[t, p, r, q, n] = h2[t][n, r*512 + q*128 + p]  (host scales by 0.5)
    out2 = nc.dram_tensor("out2", [T, 128, 2, 4, NL], F32, kind="ExternalOutput").ap()
    # xps[t, p, r, q, j, n] = xproj[t][n, j*1024 + r*512 + q*128 + p]  (bf16)
    xps = nc.dram_tensor("xps", [T, 128, 2, 4, 4, NL], BF16, kind="Internal").ap()

    e16_np, m32_np, md4_np, eye_np, eye32_np, sel4_np = _consts()
    e16_d = nc.inline_tensor(e16_np, "c_e16")
    m32_d = nc.inline_tensor(m32_np, "c_m32")
    md4_d = nc.inline_tensor(md4_np, "c_md4")
    eye_d = nc.inline_tensor(eye_np, "c_eyebf")
    eye32_d = nc.inline_tensor(eye32_np, "c_eye32")
    sel4_d = nc.inline_tensor(sel4_np, "c_sel4")

    with TileContext(nc) as tc:
        with tc.tile_pool(name="pers", bufs=1) as pers:
            Wh_sb = pers.tile([128, HC * H4], BF16, tag="Wh")
            Asc_sb = pers.tile([128, HC * 512], BF16, tag="Asc")
            P_sb = pers.tile([128, G * H4], BF16, tag="P")
            uThA = pers.tile([128, 128], BF16, tag="uThA")
            uThB = pers.tile([128, 128], BF16, tag="uThB")
            cT = pers.tile([128, 256], F32, tag="cT")
            eye = pers.tile([128, 128], BF16, tag="eye")
            eye32 = pers.tile([128, 128], F32, tag="eye32")
            sel4 = pers.tile([128, 32], F32, tag="sel4")
            E16 = pers.tile([16, 128], BF16, tag="E16")
            M32 = pers.tile([128, 128], BF16, tag="M32")
            Mdiag4 = pers.tile([128, 512], F32, tag="Mdiag4")
            b_sb = pers.tile([128, 32], F32, tag="bT")
            wsq = pers.tile([32, 32], BF16, tag="wsq")

            nc.sync.dma_start(eye[:], eye_d.ap()[:])
            nc.sync.dma_start(eye32[:], eye32_d.ap()[:])
            nc.sync.dma_start(sel4[:], sel4_d.ap()[:])
            nc.sync.dma_start(E16[:], e16_d.ap()[:])
            nc.sync.dma_start(M32[:], m32_d.ap()[:])
            nc.sync.dma_start(Mdiag4[:], md4_d.ap()[:])
            nc.sync.dma_start(b_sb[:], ap_bT[:])
            nc.gpsimd.memset(wsq[:], 0.0)
            nc.sync.dma_start(Wh_sb[:], ap_Wh[:])
            nc.sync.dma_start(Asc_sb[:], ap_Asc[:])

            # ------------- phase B: P precompute + h0/c0 init -------------
            with tc.tile_pool(name="php", bufs=3) as php, \
                 tc.tile_pool(name="php1", bufs=1) as php1, \
                 tc.tile_pool(name="psP", bufs=2, space="PSUM") as psP:
                for c in range(HC):
                    h0s = php.tile([128, 32], F32, tag="h0s")
                    nc.vector.tensor_reduce(
                        h0s[:],
                        Asc_sb[:, c * 512 : (c + 1) * 512].rearrange(
                            "p (k n) -> p n k", k=16
                        ),
                        axis=AX.X,
                        op=ALU.add,
                    )
                    nc.vector.tensor_scalar_mul(
                        cT[:, 32 * c : 32 * (c + 1)], h0s[:], 1.0 / 16.0
                    )
                    # uTh holds h2 = 2h -> init 2/16
                    uT = uThA if c < 4 else uThB
                    nc.vector.tensor_scalar_mul(
                        uT[:, 32 * (c % 4) : 32 * (c % 4 + 1)], h0s[:], 2.0 / 16.0
                    )
                # contiguous staging of the group-selected A columns so the
                # matmul stationary operand has a single free dim
                Ag = php1.tile([128, G * HC * 128], BF16, tag="Ag")
                for g in range(G):
                    for c in range(HC):
                        nc.vector.tensor_copy(
                            Ag[:, (g * HC + c) * 128 : (g * HC + c) * 128 + 128],
                            Asc_sb[:, c * 512 : (c + 1) * 512].rearrange(
                                "p (k n) -> p k n", k=16
                            )[:, :, GS * g : GS * (g + 1)],
                        )
                for blk in range(8):
                    Wab = php.tile([128, HC * 512], BF16, tag="Wab")
                    nc.sync.dma_start(
                        Wab[:], ap_Wa[:, blk * HC * 512 : (blk + 1) * HC * 512]
                    )
                    for g in range(G):
                        psp = psP.tile([128, 512], F32, tag="psp")
                        for c in range(HC):
                            nc.tensor.matmul(
                                psp[:],
                                Ag[:, (g * HC + c) * 128 : (g * HC + c) * 128 + 128],
                                Wab[:, c * 512 : (c + 1) * 512],
                                start=(c == 0),
                                stop=(c == HC - 1),
                            )
                        nc.scalar.copy(
                            P_sb[:, g * H4 + 512 * blk : g * H4 + 512 * (blk + 1)],
                            psp[:],
                        )

            # ---------------- phase A: xproj -> DRAM scratch ----------------
            with tc.tile_pool(name="phx1", bufs=1) as phx1, \
                 tc.tile_pool(name="phx", bufs=3) as phx, \
                 tc.tile_pool(name="psX", bufs=2, space="PSUM") as psX:
                xT_sb = phx1.tile([128, HC * T * NL], BF16, tag="xTsb")
                nc.sync.dma_start(xT_sb[:], ap_xT[:])
                for W in range(32):
                    j, r, q = W // 8, (W % 8) // 4, W % 4
                    Wxb = phx.tile([128, HC * 128], BF16, tag="Wxb")
                    nc.sync.dma_start(
                        Wxb[:], ap_Wx[:, W * HC * 128 : (W + 1) * HC * 128]
                    )
                    sxp4 = phx.tile([128, T * NL], BF16, tag="sxp4")
                    for t4 in range(4):
                        psx = psX.tile([128, 512], F32, tag="psx")
                        for c in range(HC):
                            nc.tensor.matmul(
                                psx[:],
                                Wxb[:, c * 128 : (c + 1) * 128],
                                xT_sb[:, c * T * NL + 512 * t4 : c * T * NL + 512 * (t4 + 1)],
                                start=(c == 0),
                                stop=(c == HC - 1),
                            )
                        nc.vector.tensor_scalar_add(
                            sxp4[:, 512 * t4 : 512 * (t4 + 1)], psx[:],
                            b_sb[:, W : W + 1],
                        )
                    # descriptor-heavy scatter writes: round-robin over the
                    # otherwise-idle gpsimd/scalar queues
                    weng = nc.gpsimd if (W % 2 == 0) else nc.scalar
                    weng.dma_start(
                        xps[:, :, r, q, j, :].transpose([1, 0, 2]),
                        sxp4[:].rearrange("p (t n) -> p t n", t=T),
                    )

            # ---------------------- phase C: recurrence ----------------------
            with tc.tile_pool(name="wrk", bufs=2) as wrk, \
                 tc.tile_pool(name="psc", bufs=2, space="PSUM") as psc_pool, \
                 tc.tile_pool(name="pwx", bufs=1, space="PSUM") as pwx_pool, \
                 tc.tile_pool(name="pstr", bufs=1, space="PSUM") as pstr_pool, \
                 tc.tile_pool(name="paT", bufs=2, space="PSUM") as paT_pool:
                for t in range(t_steps):
                    # prefetched xproj slice: [128, (r, q, j, n)] bf16
                    xpt = wrk.tile([128, 1024], BF16, tag="xpt", name=f"xpt_{t}")
                    nc.sync.dma_start(
                        xpt[:].rearrange("p (r q j n) -> p r q j n", r=2, q=4, j=4),
                        xps[t],
                    )

                    # -- scores: col-tiled cross-sample products, diag, softmax
                    psc4 = psc_pool.tile([128, 512], F32, tag="psc4")
                    for c in range(HC):
                        jj, e = c % 4, c // 4
                        uT = uThA if c < 4 else uThB
                        nc.tensor.matmul(
                            psc4[32 * jj : 32 * (jj + 1), :],
                            uT[:, 32 * (c % 4) : 32 * (c % 4) + 32],
                            Asc_sb[:, c * 512 : (c + 1) * 512],
                            start=(e == 0),
                            stop=(e == 1),
                            skip_group_check=True,
                            tile_position=(0, 32 * jj),
                        )
                    scm4 = wrk.tile([128, 512], F32, tag="scm4")
                    nc.vector.tensor_mul(scm4[:], psc4[:], Mdiag4[:])
                    red4 = wrk.tile([128, 16], F32, tag="red4")
                    nc.vector.tensor_reduce(
                        red4[:],
                        scm4[:].rearrange("p (k n) -> p k n", k=16),
                        axis=AX.X,
                        op=ALU.add,
                    )
                    scores = pwx_pool.tile([32, 16], F32, tag="scps",
                                           name=f"scps_{t}")
                    nc.tensor.matmul(
                        scores[:], sel4[:], red4[:], start=True, stop=True
                    )
                    # no max-subtraction: |scores| <~ 2 here, exp is safe in f32
                    ex = wrk.tile([32, 16], F32, tag="ex")
                    esum = wrk.tile([32, 1], F32, tag="esum")
                    nc.scalar.activation(
                        ex[:], scores[:], AF.Exp, accum_out=esum[:],
                    )
                    rcp = wrk.tile([32, 1], F32, tag="rcp")
                    nc.vector.reciprocal(rcp[:], esum[:])
                    nc.vector.tensor_scalar_mul(wsq[:, 0:16], ex[:], rcp[:])
                    wT = wrk.tile([32, 32], BF16, tag="wT")
                    nc.vector.transpose(wT[:], wsq[:])
                    pwx = pwx_pool.tile([128, 32], F32, tag="pwx")
                    nc.tensor.matmul(
                        pwx[:], E16[:], wT[0:16, 0:32], start=True, stop=True
                    )
                    masked = wrk.tile([128, 128], BF16, tag="masked")
                    nc.vector.tensor_mul(
                        masked[:].rearrange("p (g m) -> p g m", g=4),
                        pwx[:].rearrange("p (x m) -> p x m", x=1).broadcast_to(
                            [128, 4, 32]
                        ),
                        M32[:].rearrange("p (g m) -> p g m", g=4),
                    )

                    # -- gates: h2 @ Wh' + sum_k w_k P'_k, one strip per r.
                    # PE emission order keeps the array busy while each r's
                    # ACT/DVE tail runs: Wh0+P0, Wh1 (sg0 copies on Scalar),
                    # inject+transpose 0, P1 (cell 0 on DVE), inject+
                    # transpose 1 (cell 1 overlaps next step's scores).
                    h2all = wrk.tile([128, 256], F32, tag="h2all", name=f"h2_{t}")

                    def wh_block(strip, r):
                        for c in range(HC):
                            uT = uThA if c < 4 else uThB
                            for jj in range(4):
                                nc.tensor.matmul(
                                    strip[32 * jj : 32 * (jj + 1), :],
                                    uT[:, 32 * (c % 4) : 32 * (c % 4) + 32],
                                    Wh_sb[:, c * H4 + jj * 1024 + r * 512 : c * H4 + jj * 1024 + r * 512 + 512],
                                    start=(c == 0),
                                    stop=False,
                                    skip_group_check=True,
                                    tile_position=(0, 32 * jj),
                                )

                    def p_block(strip, r):
                        for g in range(G):
                            for jj in range(4):
                                nc.tensor.matmul(
                                    strip[32 * jj : 32 * (jj + 1), :],
                                    masked[:, 32 * g : 32 * g + 32],
                                    P_sb[:, g * H4 + jj * 1024 + r * 512 : g * H4 + jj * 1024 + r * 512 + 512],
                                    start=False,
                                    stop=(g == G - 1),
                                    skip_group_check=True,
                                    tile_position=(0, 32 * jj),
                                )

                    def sg_copy(strip, r):
                        sg = wrk.tile([128, 512], F32, tag=f"sg{r}")
                        nc.scalar.copy(sg[:], strip[:])
                        return sg

                    def transpose_block(sg, r):
                        pat = paT_pool.tile([128, 512], F32, tag="pat",
                                            name=f"pat{r}_{t}")
                        nc.tensor.matmul(
                            pat[:], eye[:],
                            xpt[:, r * 512 : (r + 1) * 512],
                            start=True, stop=False,
                        )
                        for q in range(4):
                            nc.tensor.matmul(
                                pat[:, 128 * q : 128 * (q + 1)],
                                sg[:, 128 * q : 128 * (q + 1)],
                                eye32[:],
                                is_transpose=True,
                                start=False,
                                stop=(q == 3),
                            )
                        return pat

                    def cell_block(pat, r):
                        # tv = tanh over the whole 512 (i/f/o pre-halved)
                        tv = wrk.tile([128, 512], F32, tag=f"tv{r}")
                        nc.scalar.activation(tv[:], pat[:], AF.Tanh)
                        tq = tv[:].rearrange("p (q j m) -> p q j m", q=4, j=4)
                        ti, tf = tq[:, :, 0, :], tq[:, :, 1, :]
                        to, tg = tq[:, :, 2, :], tq[:, :, 3, :]
                        cview = cT[:, 128 * r : 128 * (r + 1)].rearrange(
                            "p (q n) -> p q n", q=4
                        )
                        u = wrk.tile([128, 128], F32, tag=f"u{r}")
                        nc.vector.scalar_tensor_tensor(
                            u[:].rearrange("p (q n) -> p q n", q=4),
                            tf, 1.0, cview, ALU.add, ALU.mult,
                        )
                        # warm-keeper: a tiny PE op chained on the tail keeps
                        # the HAM activity window alive (else the PE
                        # re-throttles to K=4/8 once per step)
                        nc.tensor.matmul(
                            scores[:], eye32[:, 0:32], u[:, 0:16],
                            start=True, stop=True,
                        )
                        v = wrk.tile([128, 128], F32, tag=f"v{r}")
                        nc.vector.scalar_tensor_tensor(
                            v[:].rearrange("p (q n) -> p q n", q=4),
                            ti, 1.0, tg, ALU.add, ALU.mult,
                        )
                        s2 = wrk.tile([128, 128], F32, tag=f"s2{r}")
                        nc.vector.tensor_add(s2[:], u[:], v[:])
                        nc.tensor.matmul(
                            scores[:], eye32[:, 0:32], s2[:, 0:16],
                            start=True, stop=True,
                        )
                        # c = s2/2: state halving on the Scalar engine, off the
                        # DVE spine; tanh(c) reads s2 directly via input scale
                        nc.scalar.mul(cT[:, 128 * r : 128 * (r + 1)], s2[:], 0.5)
                        th = wrk.tile([128, 128], F32, tag=f"th{r}")
                        nc.scalar.activation(
                            th[:], s2[:], AF.Tanh, scale=0.5
                        )
                        # h2 = (to + 1) * tanh(c)
                        nc.vector.scalar_tensor_tensor(
                            h2all[:, 128 * r : 128 * (r + 1)].rearrange(
                                "p (q n) -> p q n", q=4
                            ),
                            to, 1.0, th[:].rearrange("p (q n) -> p q n", q=4),
                            ALU.add, ALU.mult,
                        )
                        uT = uThA if r == 0 else uThB
                        nc.vector.tensor_copy(
                            uT[:], h2all[:, 128 * r : 128 * (r + 1)]
                        )

                    strip0 = pstr_pool.tile([128, 512], F32, tag="strip0",
                                            name=f"strip0_{t}")
                    strip1 = pstr_pool.tile([128, 512], F32, tag="strip1",
                                            name=f"strip1_{t}")
                    strips = [strip0, strip1]
                    # interleave the two strips' Wh accumulation (v2 structure:
                    # keeps the PE stream dense), then the P accumulation, then
                    # the per-r tails
                    for c in range(HC):
                        uT = uThA if c < 4 else uThB
                        for r in range(2):
                            for jj in range(4):
                                nc.tensor.matmul(
                                    strips[r][32 * jj : 32 * (jj + 1), :],
                                    uT[:, 32 * (c % 4) : 32 * (c % 4) + 32],
                                    Wh_sb[:, c * H4 + jj * 1024 + r * 512 : c * H4 + jj * 1024 + r * 512 + 512],
                                    start=(c == 0),
                                    stop=False,
                                    skip_group_check=True,
                                    tile_position=(0, 32 * jj),
                                )
                    for r in range(2):
                        p_block(strips[r], r)
                    sg0 = sg_copy(strip0, 0)
                    sg1 = sg_copy(strip1, 1)
                    pat0 = transpose_block(sg0, 0)
                    pat1 = transpose_block(sg1, 1)
                    cell_block(pat0, 0)
                    cell_block(pat1, 1)
                    nc.sync.dma_start(
                        out2[t],
                        h2all[:].rearrange("p (r q n) -> p r q n", r=2, q=4),
                    )
    nc.compile()
    return nc


def _prep_shards(inputs):
    x = np.asarray(inputs["x"], np.float32)
    A = np.asarray(inputs["A"], np.float32)
    Wx = np.asarray(inputs["Wx"], np.float32)
    Wh = np.asarray(inputs["Wh"], np.float32)
    Wattn = np.asarray(inputs["Wattn"], np.float32)
    b = np.asarray(inputs["b"], np.float32)

    # tanh-only gate scaling: i/f/o columns get the sigmoid 1/2 arg-scale;
    # everything fed by h2 = 2h gets an extra 1/2.
    sc_ifo = np.ones((H4,), np.float32)
    sc_ifo[: 3 * H] = 0.5
    Wx_s = Wx * sc_ifo
    b_s = b * sc_ifo
    Wa_s = Wattn * sc_ifo
    Wh_s = Wh * (0.5 * sc_ifo)

    def chunk_rows(M, free):
        # [1024, F] -> [128, HC * F] with the 8 row-chunks along free
        return np.ascontiguousarray(
            M.reshape(HC, 128, free).transpose(1, 0, 2).reshape(128, HC * free)
        )

    Wh_bf = chunk_rows(Wh_s.astype(BF), H4)
    # WxP: [128, (W, c, 128)]
    WxP = np.ascontiguousarray(
        Wx_s.astype(BF)
        .reshape(HC, 128, 32, 128)
        .transpose(1, 2, 0, 3)
        .reshape(128, 32 * HC * 128)
    )
    # WaP: [128, (blk, c, 512)]
    WaP = np.ascontiguousarray(
        Wa_s.astype(BF)
        .reshape(HC, 128, 8, 512)
        .transpose(1, 2, 0, 3)
        .reshape(128, 8 * HC * 512)
    )
    bT = np.ascontiguousarray(b_s.reshape(32, 128).T.astype(np.float32))

    in_maps = []
    for i in range(NCORES):
        ns = slice(NL * i, NL * (i + 1))
        xT = x[ns].transpose(2, 1, 0).reshape(D, T * NL)
        xTP = chunk_rows(np.ascontiguousarray(xT).astype(BF), T * NL)
        Asc = A[ns].reshape(NL, H, 16).transpose(1, 2, 0).reshape(H, 512)
        AscP = chunk_rows(np.ascontiguousarray(Asc).astype(BF), 512)
        in_maps.append(
            {
                "xTP": xTP,
                "AscP": AscP,
                "WxP": WxP,
                "WhP": Wh_bf,
                "WaP": WaP,
                "bT": bT,
            }
        )
    return in_maps


def _get_nc():
    global _built
    if _built is None:
        _built = _build_nc()
    return _built


def _run(inputs, **kwargs):
    nc = _get_nc()
    in_maps = _prep_shards(inputs)
    res = bass_utils.run_bass_kernel_spmd(
        nc, in_maps, core_ids=list(range(NCORES)), **kwargs
    )
    out = np.empty((N, T, H), np.float32)
    for i in range(NCORES):
        o2 = res.results[i]["out2"]  # [T, 128, 2, 4, NL], h2 values
        out[NL * i : NL * (i + 1)] = 0.5 * o2.transpose(4, 0, 2, 3, 1).reshape(
            NL, T, H
        )
    return out, res


def kernel(**inputs):
    out, _ = _run(inputs)
    return out


# revision 32
# speedup vs baseline: 1.1208x; 1.0019x over previous
"""Trainium2 Bass kernel for nn_CaptioningRNN (attention LSTM over T=64).

Data-parallel over the batch: N=256 samples split across 8 NeuronCores
(32 samples/core), weights replicated, no collectives.

v2 structure (all matmuls bf16 on the TensorEngine, state in f32):
  - tanh-only gates: sigmoid(x) = (tanh(x/2)+1)/2 with the 1/2 folded into
    host-side weight pre-scaling, and h tracked as h2 = 2h (Wh pre-scaled by
    an extra 1/2, score scale halved, output halved on the host). exp and
    tanh live in the same ACT table -> no ACT_TABLE_LOAD per step.
  - xproj phase: xpT = x @ Wx + b computed Wx-stationary, stored to a DRAM
    scratch in bf16 laid out [T, 128, 2, (q,j,n)] so the per-step slice is
    one clean [128, 1024] DMA and doubles as the moving operand of an
    identity-stationary matmul that injects xproj into the transposed-gates
    PSUM accumulation (no DVE adds).
  - P phase: P[n,k,:] = A[n,:,k] @ Wattn precomputed once; h0 = c0 = mean.
  - Recurrence (64 steps): col-tiled scores (4 groups x 2 chunks) ->
    mask+reduce diag extract -> softmax (exp) -> 32x32 transpose -> one-hot
    matmul + broadcast mask mul -> block-diag w; gates = h2 @ Wh' +
    sum_k w_k P'_k in 2 column-tiled PSUM strips (shared stationary across
    the 4 col-groups); strips cast to bf16 on the Scalar engine, transposed
    on PE with the xproj inject; one Tanh per strip; cell math in
    h-on-partition space with the sigmoid affine folded in.
  - Output written [t][p][(r,q,n)] f32; host reassembles + scales by 0.5.
"""

import numpy as np
import ml_dtypes

import concourse.bacc as bacc
import concourse.mybir as mybir
from concourse import bass_utils
from concourse.tile import TileContext

F32, BF16 = mybir.dt.float32, mybir.dt.bfloat16
AF = mybir.ActivationFunctionType
ALU = mybir.AluOpType
AX = mybir.AxisListType
BF = ml_dtypes.bfloat16

N, T, D, H = 256, 64, 1024, 1024
NCORES = 8
NL = N // NCORES          # 32 samples per core
HC = 8                    # 128-row chunks of D/H
G, GS = 4, 8              # sample groups of 8 (for the (k, n_g) 128-partition layout)
H4 = 4 * H                # 4096 gate columns

_built = None


def _consts():
    # E16[k', 8k + n] = (k' == k): one-hot expansion of wT rows onto the
    # (k-major, n_g-minor) 128-partition layout.
    e16 = np.zeros((16, 128), dtype=BF)
    for k in range(16):
        e16[k, 8 * k : 8 * k + 8] = 1
    # M32[p, 32 g + m] = (m % 8 == p % 8) & (m // 8 == g): block-diagonal
    # mask producing masked_g = w[m, k(p)] only for group-g samples.
    p = np.arange(128)[:, None]
    m = np.arange(32)[None, :]
    m32 = np.zeros((128, 128), dtype=BF)
    for g in range(4):
        m32[:, 32 * g : 32 * g + 32] = ((m % 8 == p % 8) & (m // 8 == g)).astype(BF)
    # Mdiag4[32 j + n, 32 k + n'] = (n == n') / 64: extracts the diagonal of
    # the cross-sample score products; 1/64 = softmax scale 1/sqrt(H) times
    # the 1/2 compensating h2 = 2h. Tiled over the 4 col-tile groups.
    md4 = np.zeros((128, 512), dtype=np.float32)
    n_ = np.arange(32)
    for j in range(4):
        for k in range(16):
            md4[32 * j + n_, 32 * k + n_] = 1.0 / 64.0
    eye_bf = np.eye(128, dtype=BF)
    eye_f32 = np.eye(128, dtype=np.float32)
    # sel4[32 j + m, m'] = (m == m'): matmul-stationary that sums the four
    # col-tile partition groups of the partial score reductions.
    sel4 = np.tile(np.eye(32, dtype=np.float32), (4, 1))
    return e16, m32, md4, eye_bf, eye_f32, sel4


def _build_nc(t_steps=T):
    nc = bacc.Bacc(trn_type="TRN2", target_bir_lowering=False, debug=False)

    # host-packed inputs (see _prep_shards for layouts)
    ap_xT = nc.dram_tensor("xTP", [128, HC * T * NL], BF16, kind="ExternalInput").ap()
    ap_Asc = nc.dram_tensor("AscP", [128, HC * 512], BF16, kind="ExternalInput").ap()
    ap_Wx = nc.dram_tensor("WxP", [128, 32 * HC * 128], BF16, kind="ExternalInput").ap()
    ap_Wh = nc.dram_tensor("WhP", [128, HC * H4], BF16, kind="ExternalInput").ap()
    ap_Wa = nc.dram_tensor("WaP", [128, 8 * HC * 512], BF16, kind="ExternalInput").ap()
    ap_bT = nc.dram_tensor("bT", [128, 32], F32, kind="ExternalInput").ap()
    # out2[t, p, r, q, n] = h2[t][n, r*512 + q*128 + p]  (host scales by 0.5)
    out2 = nc.dram_tensor("out2", [T, 128, 2, 4, NL], BF16, kind="ExternalOutput").ap()
    # xps[t, p, r, q, j, n] = xproj[t][n, j*1024 + r*512 + q*128 + p]  (bf16)
    xps = nc.dram_tensor("xps", [T, 128, 2, 4, 4, NL], BF16, kind="Internal").ap()

    e16_np, m32_np, md4_np, eye_np, eye32_np, sel4_np = _consts()
    e16_d = nc.inline_tensor(e16_np, "c_e16")
    m32_d = nc.inline_tensor(m32_np, "c_m32")
    md4_d = nc.inline_tensor(md4_np, "c_md4")
    eye_d = nc.inline_tensor(eye_np, "c_eyebf")
    eye32_d = nc.inline_tensor(eye32_np, "c_eye32")
    sel4_d = nc.inline_tensor(sel4_np, "c_sel4")

    with TileContext(nc) as tc:
        with tc.tile_pool(name="pers", bufs=1) as pers:
            Wh_sb = pers.tile([128, HC * H4], BF16, tag="Wh")
            Asc_sb = pers.tile([128, HC * 512], BF16, tag="Asc")
            P_sb = pers.tile([128, G * H4], BF16, tag="P")
            uThA = pers.tile([128, 128], BF16, tag="uThA")
            uThB = pers.tile([128, 128], BF16, tag="uThB")
            cT = pers.tile([128, 256], F32, tag="cT")
            eye = pers.tile([128, 128], BF16, tag="eye")
            eye32 = pers.tile([128, 128], F32, tag="eye32")
            sel4 = pers.tile([128, 32], F32, tag="sel4")
            E16 = pers.tile([16, 128], BF16, tag="E16")
            M32 = pers.tile([128, 128], BF16, tag="M32")
            Mdiag4 = pers.tile([128, 512], F32, tag="Mdiag4")
            b_sb = pers.tile([128, 32], F32, tag="bT")
            wsq = pers.tile([32, 32], BF16, tag="wsq")

            nc.sync.dma_start(eye[:], eye_d.ap()[:])
            nc.sync.dma_start(eye32[:], eye32_d.ap()[:])
            nc.sync.dma_start(sel4[:], sel4_d.ap()[:])
            nc.sync.dma_start(E16[:], e16_d.ap()[:])
            nc.sync.dma_start(M32[:], m32_d.ap()[:])
            nc.sync.dma_start(Mdiag4[:], md4_d.ap()[:])
            nc.sync.dma_start(b_sb[:], ap_bT[:])
            nc.gpsimd.memset(wsq[:], 0.0)
            nc.sync.dma_start(Wh_sb[:], ap_Wh[:])
            nc.sync.dma_start(Asc_sb[:], ap_Asc[:])

            # ------------- phase B: P precompute + h0/c0 init -------------
            with tc.tile_pool(name="php", bufs=3) as php, \
                 tc.tile_pool(name="php1", bufs=1) as php1, \
                 tc.tile_pool(name="psP", bufs=2, space="PSUM") as psP:
                for c in range(HC):
                    h0s = php.tile([128, 32], F32, tag="h0s")
                    nc.vector.tensor_reduce(
                        h0s[:],
                        Asc_sb[:, c * 512 : (c + 1) * 512].rearrange(
                            "p (k n) -> p n k", k=16
                        ),
                        axis=AX.X,
                        op=ALU.add,
                    )
                    nc.vector.tensor_scalar_mul(
                        cT[:, 32 * c : 32 * (c + 1)], h0s[:], 1.0 / 16.0
                    )
                    # uTh holds h2 = 2h -> init 2/16
                    uT = uThA if c < 4 else uThB
                    nc.vector.tensor_scalar_mul(
                        uT[:, 32 * (c % 4) : 32 * (c % 4 + 1)], h0s[:], 2.0 / 16.0
                    )
                # contiguous staging of the group-selected A columns so the
                # matmul stationary operand has a single free dim
                Ag = php1.tile([128, G * HC * 128], BF16, tag="Ag")
                for g in range(G):
                    for c in range(HC):
                        nc.vector.tensor_copy(
                            Ag[:, (g * HC + c) * 128 : (g * HC + c) * 128 + 128],
                            Asc_sb[:, c * 512 : (c + 1) * 512].rearrange(
                                "p (k n) -> p k n", k=16
                            )[:, :, GS * g : GS * (g + 1)],
                        )
                for blk in range(8):
                    Wab = php.tile([128, HC * 512], BF16, tag="Wab")
                    nc.sync.dma_start(
                        Wab[:], ap_Wa[:, blk * HC * 512 : (blk + 1) * HC * 512]
                    )
                    for g in range(G):
                        psp = psP.tile([128, 512], F32, tag="psp")
                        for c in range(HC):
                            nc.tensor.matmul(
                                psp[:],
                                Ag[:, (g * HC + c) * 128 : (g * HC + c) * 128 + 128],
                                Wab[:, c * 512 : (c + 1) * 512],
                                start=(c == 0),
                                stop=(c == HC - 1),
                            )
                        nc.scalar.copy(
                            P_sb[:, g * H4 + 512 * blk : g * H4 + 512 * (blk + 1)],
                            psp[:],
                        )

            # ---------------- phase A: xproj -> DRAM scratch ----------------
            with tc.tile_pool(name="phx1", bufs=1) as phx1, \
                 tc.tile_pool(name="phx", bufs=3) as phx, \
                 tc.tile_pool(name="psX", bufs=2, space="PSUM") as psX:
                xT_sb = phx1.tile([128, HC * T * NL], BF16, tag="xTsb")
                nc.sync.dma_start(xT_sb[:], ap_xT[:])
                for W in range(32):
                    j, r, q = W // 8, (W % 8) // 4, W % 4
                    Wxb = phx.tile([128, HC * 128], BF16, tag="Wxb")
                    nc.sync.dma_start(
                        Wxb[:], ap_Wx[:, W * HC * 128 : (W + 1) * HC * 128]
                    )
                    sxp4 = phx.tile([128, T * NL], BF16, tag="sxp4")
                    for t4 in range(4):
                        psx = psX.tile([128, 512], F32, tag="psx")
                        for c in range(HC):
                            nc.tensor.matmul(
                                psx[:],
                                Wxb[:, c * 128 : (c + 1) * 128],
                                xT_sb[:, c * T * NL + 512 * t4 : c * T * NL + 512 * (t4 + 1)],
                                start=(c == 0),
                                stop=(c == HC - 1),
                            )
                        nc.vector.tensor_scalar_add(
                            sxp4[:, 512 * t4 : 512 * (t4 + 1)], psx[:],
                            b_sb[:, W : W + 1],
                        )
                    # descriptor-heavy scatter writes: round-robin over the
                    # otherwise-idle gpsimd/scalar queues
                    weng = nc.gpsimd if (W % 2 == 0) else nc.scalar
                    weng.dma_start(
                        xps[:, :, r, q, j, :].transpose([1, 0, 2]),
                        sxp4[:].rearrange("p (t n) -> p t n", t=T),
                    )

            # ---------------------- phase C: recurrence ----------------------
            with tc.tile_pool(name="wrk", bufs=2) as wrk, \
                 tc.tile_pool(name="psc", bufs=2, space="PSUM") as psc_pool, \
                 tc.tile_pool(name="pwx", bufs=1, space="PSUM") as pwx_pool, \
                 tc.tile_pool(name="pstr", bufs=1, space="PSUM") as pstr_pool, \
                 tc.tile_pool(name="paT", bufs=2, space="PSUM") as paT_pool:
                for t in range(t_steps):
                    # prefetched xproj slice: [128, (r, q, j, n)] bf16
                    xpt = wrk.tile([128, 1024], BF16, tag="xpt", name=f"xpt_{t}")
                    nc.sync.dma_start(
                        xpt[:].rearrange("p (r q j n) -> p r q j n", r=2, q=4, j=4),
                        xps[t],
                    )

                    # -- scores: col-tiled cross-sample products, diag, softmax
                    psc4 = psc_pool.tile([128, 512], F32, tag="psc4")
                    for c in range(HC):
                        jj, e = c % 4, c // 4
                        uT = uThA if c < 4 else uThB
                        nc.tensor.matmul(
                            psc4[32 * jj : 32 * (jj + 1), :],
                            uT[:, 32 * (c % 4) : 32 * (c % 4) + 32],
                            Asc_sb[:, c * 512 : (c + 1) * 512],
                            start=(e == 0),
                            stop=(e == 1),
                            skip_group_check=True,
                            tile_position=(0, 32 * jj),
                        )
                    scm4 = wrk.tile([128, 512], F32, tag="scm4")
                    nc.vector.tensor_mul(scm4[:], psc4[:], Mdiag4[:])
                    red4 = wrk.tile([128, 16], F32, tag="red4")
                    nc.vector.tensor_reduce(
                        red4[:],
                        scm4[:].rearrange("p (k n) -> p k n", k=16),
                        axis=AX.X,
                        op=ALU.add,
                    )
                    scores = pwx_pool.tile([32, 16], F32, tag="scps",
                                           name=f"scps_{t}")
                    nc.tensor.matmul(
                        scores[:], sel4[:], red4[:], start=True, stop=True
                    )
                    # no max-subtraction: |scores| <~ 2 here, exp is safe in f32
                    ex = wrk.tile([32, 16], F32, tag="ex")
                    esum = wrk.tile([32, 1], F32, tag="esum")
                    nc.scalar.activation(
                        ex[:], scores[:], AF.Exp, accum_out=esum[:],
                    )
                    rcp = wrk.tile([32, 1], F32, tag="rcp")
                    nc.vector.reciprocal(rcp[:], esum[:])
                    nc.vector.tensor_scalar_mul(wsq[:, 0:16], ex[:], rcp[:])
                    wT = wrk.tile([32, 32], BF16, tag="wT")
                    nc.vector.transpose(wT[:], wsq[:])
                    pwx = pwx_pool.tile([128, 32], F32, tag="pwx")
                    nc.tensor.matmul(
                        pwx[:], E16[:], wT[0:16, 0:32], start=True, stop=True
                    )
                    masked = wrk.tile([128, 128], BF16, tag="masked")
                    nc.vector.tensor_mul(
                        masked[:].rearrange("p (g m) -> p g m", g=4),
                        pwx[:].rearrange("p (x m) -> p x m", x=1).broadcast_to(
                            [128, 4, 32]
                        ),
                        M32[:].rearrange("p (g m) -> p g m", g=4),
                    )

                    # -- gates: h2 @ Wh' + sum_k w_k P'_k, one strip per r.
                    # PE emission order keeps the array busy while each r's
                    # ACT/DVE tail runs: Wh0+P0, Wh1 (sg0 copies on Scalar),
                    # inject+transpose 0, P1 (cell 0 on DVE), inject+
                    # transpose 1 (cell 1 overlaps next step's scores).
                    h2all = wrk.tile([128, 256], BF16, tag="h2all", name=f"h2_{t}")

                    def wh_block(strip, r):
                        for c in range(HC):
                            uT = uThA if c < 4 else uThB
                            for jj in range(4):
                                nc.tensor.matmul(
                                    strip[32 * jj : 32 * (jj + 1), :],
                                    uT[:, 32 * (c % 4) : 32 * (c % 4) + 32],
                                    Wh_sb[:, c * H4 + jj * 1024 + r * 512 : c * H4 + jj * 1024 + r * 512 + 512],
                                    start=(c == 0),
                                    stop=False,
                                    skip_group_check=True,
                                    tile_position=(0, 32 * jj),
                                )

                    def p_block(strip, r):
                        for g in range(G):
                            for jj in range(4):
                                nc.tensor.matmul(
                                    strip[32 * jj : 32 * (jj + 1), :],
                                    masked[:, 32 * g : 32 * g + 32],
                                    P_sb[:, g * H4 + jj * 1024 + r * 512 : g * H4 + jj * 1024 + r * 512 + 512],
                                    start=False,
                                    stop=(g == G - 1),
                                    skip_group_check=True,
                                    tile_position=(0, 32 * jj),
                                )

                    def sg_copy(strip, r):
                        sg = wrk.tile([128, 512], F32, tag=f"sg{r}")
                        nc.scalar.copy(sg[:], strip[:])
                        return sg

                    def transpose_block(sg, r):
                        pat = paT_pool.tile([128, 512], F32, tag="pat",
                                            name=f"pat{r}_{t}")
                        nc.tensor.matmul(
                            pat[:], eye[:],
                            xpt[:, r * 512 : (r + 1) * 512],
                            start=True, stop=False,
                        )
                        for q in range(4):
                            nc.tensor.matmul(
                                pat[:, 128 * q : 128 * (q + 1)],
                                sg[:, 128 * q : 128 * (q + 1)],
                                eye32[:],
                                is_transpose=True,
                                start=False,
                                stop=(q == 3),
                            )
                        return pat

                    def cell_block(pat, r):
                        # tv = tanh over the whole 512 (i/f/o pre-halved)
                        tv = wrk.tile([128, 512], F32, tag=f"tv{r}")
                        nc.scalar.activation(tv[:], pat[:], AF.Tanh)
                        tq = tv[:].rearrange("p (q j m) -> p q j m", q=4, j=4)
                        ti, tf = tq[:, :, 0, :], tq[:, :, 1, :]
                        to, tg = tq[:, :, 2, :], tq[:, :, 3, :]
                        cview = cT[:, 128 * r : 128 * (r + 1)].rearrange(
                            "p (q n) -> p q n", q=4
                        )
                        u = wrk.tile([128, 128], F32, tag=f"u{r}")
                        nc.vector.scalar_tensor_tensor(
                            u[:].rearrange("p (q n) -> p q n", q=4),
                            tf, 1.0, cview, ALU.add, ALU.mult,
                        )
                        # warm-keeper: a tiny PE op chained on the tail keeps
                        # the HAM activity window alive (else the PE
                        # re-throttles to K=4/8 once per step)
                        nc.tensor.matmul(
                            scores[:], eye32[:, 0:32], u[:, 0:16],
                            start=True, stop=True,
                        )
                        v = wrk.tile([128, 128], F32, tag=f"v{r}")
                        nc.vector.scalar_tensor_tensor(
                            v[:].rearrange("p (q n) -> p q n", q=4),
                            ti, 1.0, tg, ALU.add, ALU.mult,
                        )
                        s2 = wrk.tile([128, 128], F32, tag=f"s2{r}")
                        nc.vector.tensor_add(s2[:], u[:], v[:])
                        nc.tensor.matmul(
                            scores[:], eye32[:, 0:32], s2[:, 0:16],
                            start=True, stop=True,
                        )
                        # c = s2/2: state halving on the Scalar engine, off the
                        # DVE spine; tanh(c) reads s2 directly via input scale
                        nc.scalar.mul(cT[:, 128 * r : 128 * (r + 1)], s2[:], 0.5)
                        th = wrk.tile([128, 128], F32, tag=f"th{r}")
                        nc.scalar.activation(
                            th[:], s2[:], AF.Tanh, scale=0.5
                        )
                        # h2 = (to + 1) * tanh(c)
                        nc.vector.scalar_tensor_tensor(
                            h2all[:, 128 * r : 128 * (r + 1)].rearrange(
                                "p (q n) -> p q n", q=4
                            ),
                            to, 1.0, th[:].rearrange("p (q n) -> p q n", q=4),
                            ALU.add, ALU.mult,
                        )
                        uT = uThA if r == 0 else uThB
                        nc.vector.tensor_copy(
                            uT[:], h2all[:, 128 * r : 128 * (r + 1)]
                        )
                        nc.tensor.matmul(
                            scores[:], eye32[:, 0:32], th[:, 0:16],
                            start=True, stop=True,
                        )

                    strip0 = pstr_pool.tile([128, 512], F32, tag="strip0",
                                            name=f"strip0_{t}")
                    strip1 = pstr_pool.tile([128, 512], F32, tag="strip1",
                                            name=f"strip1_{t}")
                    strips = [strip0, strip1]
                    # interleave the two strips' Wh accumulation (v2 structure:
                    # keeps the PE stream dense), then the P accumulation, then
                    # the per-r tails
                    for c in range(HC):
                        uT = uThA if c < 4 else uThB
                        for r in range(2):
                            for jj in range(4):
                                nc.tensor.matmul(
                                    strips[r][32 * jj : 32 * (jj + 1), :],
                                    uT[:, 32 * (c % 4) : 32 * (c % 4) + 32],
                                    Wh_sb[:, c * H4 + jj * 1024 + r * 512 : c * H4 + jj * 1024 + r * 512 + 512],
                                    start=(c == 0),
                                    stop=False,
                                    skip_group_check=True,
                                    tile_position=(0, 32 * jj),
                                )
                    for r in range(2):
                        p_block(strips[r], r)
                    sg0 = sg_copy(strip0, 0)
                    sg1 = sg_copy(strip1, 1)
                    pat0 = transpose_block(sg0, 0)
                    pat1 = transpose_block(sg1, 1)
                    cell_block(pat0, 0)
                    cell_block(pat1, 1)
                    nc.sync.dma_start(
                        out2[t],
                        h2all[:].rearrange("p (r q n) -> p r q n", r=2, q=4),
                    )
    nc.compile()
    return nc


def _prep_shards(inputs):
    x = np.asarray(inputs["x"], np.float32)
    A = np.asarray(inputs["A"], np.float32)
    Wx = np.asarray(inputs["Wx"], np.float32)
    Wh = np.asarray(inputs["Wh"], np.float32)
    Wattn = np.asarray(inputs["Wattn"], np.float32)
    b = np.asarray(inputs["b"], np.float32)

    # tanh-only gate scaling: i/f/o columns get the sigmoid 1/2 arg-scale;
    # everything fed by h2 = 2h gets an extra 1/2.
    sc_ifo = np.ones((H4,), np.float32)
    sc_ifo[: 3 * H] = 0.5
    Wx_s = Wx * sc_ifo
    b_s = b * sc_ifo
    Wa_s = Wattn * sc_ifo
    Wh_s = Wh * (0.5 * sc_ifo)

    def chunk_rows(M, free):
        # [1024, F] -> [128, HC * F] with the 8 row-chunks along free
        return np.ascontiguousarray(
            M.reshape(HC, 128, free).transpose(1, 0, 2).reshape(128, HC * free)
        )

    Wh_bf = chunk_rows(Wh_s.astype(BF), H4)
    # WxP: [128, (W, c, 128)]
    WxP = np.ascontiguousarray(
        Wx_s.astype(BF)
        .reshape(HC, 128, 32, 128)
        .transpose(1, 2, 0, 3)
        .reshape(128, 32 * HC * 128)
    )
    # WaP: [128, (blk, c, 512)]
    WaP = np.ascontiguousarray(
        Wa_s.astype(BF)
        .reshape(HC, 128, 8, 512)
        .transpose(1, 2, 0, 3)
        .reshape(128, 8 * HC * 512)
    )
    bT = np.ascontiguousarray(b_s.reshape(32, 128).T.astype(np.float32))

    in_maps = []
    for i in range(NCORES):
        ns = slice(NL * i, NL * (i + 1))
        xT = x[ns].transpose(2, 1, 0).reshape(D, T * NL)
        xTP = chunk_rows(np.ascontiguousarray(xT).astype(BF), T * NL)
        Asc = A[ns].reshape(NL, H, 16).transpose(1, 2, 0).reshape(H, 512)
        AscP = chunk_rows(np.ascontiguousarray(Asc).astype(BF), 512)
        in_maps.append(
            {
                "xTP": xTP,
                "AscP": AscP,
                "WxP": WxP,
                "WhP": Wh_bf,
                "WaP": WaP,
                "bT": bT,
            }
        )
    return in_maps


def _get_nc():
    global _built
    if _built is None:
        _built = _build_nc()
    return _built


def _run(inputs, **kwargs):
    nc = _get_nc()
    in_maps = _prep_shards(inputs)
    res = bass_utils.run_bass_kernel_spmd(
        nc, in_maps, core_ids=list(range(NCORES)), **kwargs
    )
    out = np.empty((N, T, H), np.float32)
    for i in range(NCORES):
        o2 = res.results[i]["out2"].astype(np.float32)  # [T,128,2,4,NL], h2
        out[NL * i : NL * (i + 1)] = 0.5 * o2.transpose(4, 0, 2, 3, 1).reshape(
            NL, T, H
        )
    return out, res


def kernel(**inputs):
    out, _ = _run(inputs)
    return out
